# revision 1
# baseline (speedup 1.0000x reference)
"""Multi-head self-attention (2D RoPE) Trainium2 Bass kernel.

Problem: x[4,512,64,64], w_qkv[1536,512], w_proj[512,512], 8 heads, hd=64,
N=4096 positions.  out = proj(attn(rope(q), rope(k)) @ v).

Sharding (8 cores): core c -> batch b=c//2, head-group g=c%2 (heads 4g..4g+3).
Each core computes a partial projection output [512, 4096] over its 256
attention-output channels; host sums the two partials per batch (the
"all-reduce" of the tensor-parallel split) and reshapes.

Per-core kernel design:
 - QKV projection as matmul with host-transposed weights; the RoPE rotation is
   linear in q, so the rotated-pair permutation J is folded into extra weight
   columns (Jq = J@Wq, Jk = J@Wk) and RoPE becomes q*COS + (Jq x)*SIN -- three
   full-width [128, n] vector ops, no per-head slicing.
 - Attention in transposed layout: S^T[m,n] = k_m . q_n via lhsT=k^T (64
   contract rows), two heads packed in the PE array via tile_position rows
   (0,0)/(64,0).  exp() runs on the scalar engine straight out of PSUM with the
   1/8 scale folded in; no max-subtraction (|S|/8 <= ~10, fp32 exp is safe).
 - P^T needs no transpose for the PV matmul (contract dim m is already on
   partitions), and V is produced directly in [m, d] layout by an extra
   matmul X^T @ Wv^T.  The PV stationary is [ones | zeros | V] (128 cols) so the matmul emits
   softmax denominators at psum partition 0 (where the fast reciprocal and
   partition_broadcast want them) and values at partitions 64-127.
 - The whole datapath runs fp16 (inputs cast on host): fp16 streams the PE
   at full 2.4 GHz where fp32/fp32r run at half rate, and all accumulation
   stays fp32 in PSUM, so the end-to-end error remains ~1e-3.
 - q/k tiles are double-buffered across the two head pairs so pair 1's
   projection overlaps pair 0's attention (it gets a 2-bank psum aux pool
   disjoint from attention's 6 banks); the output projection runs inside
   pair 1's attention loop per 512-column chunk.
"""

import numpy as np

import concourse.bass as bass
import concourse.mybir as mybir
import concourse.tile as tile
from concourse import bacc
from concourse.bass import ts
from concourse.bass_utils import run_bass_kernel_spmd

F32 = mybir.dt.float32
F32R = mybir.dt.float32r
BF16 = mybir.dt.bfloat16
FP16 = mybir.dt.float16
AF = mybir.ActivationFunctionType

B, DIM, H, W = 4, 512, 64, 64
HEADS = 8
HD = 64
MAX_FREQ = 10000.0
N_CORES = 8

FULL = dict(N=4096, CH=512, NS=512)


def r(ap):
    return ap.bitcast(F32R)


def build_nc(N=4096, CH=512, NS=512):
    """Build the per-core Bass program (identical on all 8 cores)."""
    NMT = N // 128        # m tiles
    NCH = N // CH         # phase-1 chunks
    NNS = N // NS         # phase-3 chunks per head
    KC = DIM // 128       # contract tiles for qkv proj

    nc = bacc.Bacc("TRN2", target_bir_lowering=False, debug=False,
                   num_devices=N_CORES)

    x_d = nc.dram_tensor("x", [DIM, N], FP16, kind="ExternalInput").ap()
    wqkv_d = nc.dram_tensor("wqkvT", [DIM, 1280], FP16, kind="ExternalInput").ap()
    wv_d = nc.dram_tensor("wvT", [DIM, 256], FP16, kind="ExternalInput").ap()
    wp_d = nc.dram_tensor("wprojT", [256, DIM], FP16, kind="ExternalInput").ap()
    cos_d = nc.dram_tensor("cos", [128, N], F32, kind="ExternalInput").ap()
    sin_d = nc.dram_tensor("sin", [128, N], F32, kind="ExternalInput").ap()
    out_d = nc.dram_tensor("out", [DIM, N], F32, kind="ExternalOutput").ap()

    with tile.TileContext(nc) as tc:
        with (
            tc.tile_pool(name="singles", bufs=1) as singles,
            tc.tile_pool(name="qkpool", bufs=2) as qkpool,
            tc.tile_pool(name="xp", bufs=2) as xp,
            tc.tile_pool(name="csp", bufs=2) as csp,
            tc.tile_pool(name="ropep", bufs=2) as ropep,
            tc.tile_pool(name="ptp", bufs=8) as ptp,
            tc.tile_pool(name="nsm", bufs=6) as nsm,
            tc.tile_pool(name="ocp", bufs=8) as ocp,
            tc.tile_pool(name="osb", bufs=2) as osb,
        ):
            wq_sb = singles.tile([128, KC, 1280], FP16, tag="wq")
            for kc in range(KC):
                nc.sync.dma_start(
                    out=wq_sb[:, kc, :],
                    in_=wqkv_d[ts(kc, 128), :])
            wv_sb = singles.tile([128, KC, 256], FP16, tag="wv")
            nc.sync.dma_start(
                out=wv_sb[:],
                in_=wv_d.rearrange("(kc p) m -> p kc m", p=128))
            wp_sb = singles.tile([128, 2, DIM], FP16, tag="wp")

            v_sb = singles.tile([128, NMT, 4, 128], FP16, tag="v_sb")
            # pad lanes 1..63 of every head block are never written by the
            # compact V build; zero them once so the PV matmul streams zeros
            nc.vector.memset(v_sb[:, :, :, 0:64], 0.0)
            # one output tile PER CHUNK: a single big outT tile would create
            # whole-tile WAR deps (chunk c's normalize-write waiting chunk
            # c-1's projection-read), cascading every normalize one chunk late
            outs = [singles.tile([128, 2, NS], FP16, tag=f"outc{c}",
                                  name=f"outc{c}")
                    for c in range(NNS)]

            def phase1(p, pool, vpool):
                q_rot = qkpool.tile([128, N], FP16, tag="q_rot")
                k_rot = qkpool.tile([128, N], FP16, tag="k_rot")
                for ci in range(NCH):
                    c0 = ci * CH
                    x_t = xp.tile([128, KC, CH], FP16, tag="x_t")
                    nc.sync.dma_start(
                        out=x_t[:],
                        in_=x_d[:, c0:c0 + CH].rearrange(
                            "(kc p) n -> p kc n", p=128))
                    cos_t = csp.tile([128, CH], F32, tag="cos_t")
                    nc.sync.dma_start(out=cos_t[:], in_=cos_d[:, c0:c0 + CH])
                    sin_t = csp.tile([128, CH], F32, tag="sin_t")
                    nc.sync.dma_start(out=sin_t[:], in_=sin_d[:, c0:c0 + CH])

                    def mo_pair(mo_a, mo_b, dst):
                        ps_pair = []
                        for mo in (mo_a, mo_b):
                            ps = pool.tile([128, CH], F32, tag="aux")
                            col = p * 640 + mo * 128
                            for kc in range(KC):
                                nc.tensor.matmul(
                                    ps[:],
                                    lhsT=wq_sb[:, kc, col:col + 128],
                                    rhs=x_t[:, kc, :],
                                    start=(kc == 0), stop=(kc == KC - 1))
                            ps_pair.append(ps)
                        t1 = ropep.tile([128, CH], F32, tag="t1")
                        nc.vector.tensor_mul(t1[:], ps_pair[0][:], cos_t[:])
                        t2 = ropep.tile([128, CH], F32, tag="t2")
                        nc.vector.tensor_mul(t2[:], ps_pair[1][:], sin_t[:])
                        nc.vector.tensor_add(dst[:, c0:c0 + CH], t1[:], t2[:])

                    # k first: attention needs ALL k chunks (m dim) but only
                    # chunk ns of q, so k production is the critical path
                    mo_pair(2, 3, k_rot)
                    if p == 0:
                        # V (all 4 heads) in [m, d] layout: X^T @ Wv^T
                        for j in range(CH // 128):
                            mt = (c0 // 128) + j
                            vp = vpool.tile([128, 4, 64], F32, tag="aux")
                            for kc in range(KC):
                                nc.tensor.matmul(
                                    vp[:],
                                    lhsT=x_t[:, kc, ts(j, 128)],
                                    rhs=wv_sb[:, kc, :],
                                    start=(kc == 0), stop=(kc == KC - 1))
                            nc.vector.tensor_copy(
                                v_sb[:, mt, :, 64:128], vp[:])
                            nc.vector.memset(v_sb[:, mt, :, 0:1], 1.0)
                    mo_pair(0, 1, q_rot)
                return q_rot, k_rot

            def proj_po(n0, po, aux):
                pp = aux.tile([128, NS], F32, tag="aux")
                for ct in range(2):
                    nc.tensor.matmul(
                        pp[:],
                        lhsT=wp_sb[:, ct, ts(po, 128)],
                        rhs=outs[n0 // NS][:, ct, :],
                        start=(ct == 0), stop=(ct == 1))
                ot = osb.tile([128, NS], F32, tag="ot")
                nc.vector.tensor_copy(ot[:], pp[:])
                nc.sync.dma_start(out=out_d[ts(po, 128), n0:n0 + NS], in_=ot[:])

            def proj_chunk(n0, aux):
                # output projection for one finished 512-column chunk
                for po in range(4):
                    pp = aux.tile([128, NS], F32, tag="aux")
                    for ct in range(2):
                        nc.tensor.matmul(
                            pp[:],
                            lhsT=wp_sb[:, ct, ts(po, 128)],
                            rhs=outs[n0 // NS][:, ct, :],
                            start=(ct == 0), stop=(ct == 1))
                    ot = osb.tile([128, NS], F32, tag="ot")
                    nc.vector.tensor_copy(ot[:], pp[:])
                    nc.sync.dma_start(
                        out=out_d[ts(po, 128), n0:n0 + NS], in_=ot[:])

            def phase3(p, q_rot, k_rot, sp, oap, aux):
                LAG = min(5, NMT - 1)

                def emit_norm(pend):
                    ocs, pn0 = pend
                    for (oc, row0) in zip(ocs, (0, 64)):
                        rec = nsm.tile([1, NS], F32, tag="rec")
                        rsc = nsm.tile([1, NS], F32, tag="rsc")
                        nc.vector.reciprocal_approx_accurate(
                            rec[:], oc[0:1, :], rsc[:])
                        rb = nsm.tile([128, NS], F32, tag="rb")
                        nc.gpsimd.partition_broadcast(rb[:], rec[:])
                        nc.vector.tensor_mul(
                            outs[pn0 // NS][row0:row0 + 64, p, :],
                            oc[64:128, :], rb[64:128, :])

                # one flat software pipeline over all (ns, mt) units: QK/exp
                # run LAG units ahead of PV *continuously across chunk
                # boundaries*, so the PE FIFO never drains its PV tail and the
                # scalar engine never starves at a boundary
                total = NNS * NMT
                pts = {}
                accs = {}
                pending = None
                for u in range(total + LAG):
                    if u < total:
                        ns, mt = divmod(u, NMT)
                        n0 = ns * NS
                        s_t = sp.tile([128, 2 * NS], F32, tag="s_t")
                        nc.tensor.matmul(
                            s_t[:, 0:NS],
                            lhsT=k_rot[0:64, ts(mt, 128)],
                            rhs=q_rot[0:64, n0:n0 + NS],
                            start=True, stop=True, tile_position=(0, 0))
                        nc.tensor.matmul(
                            s_t[:, NS:2 * NS],
                            lhsT=k_rot[64:128, ts(mt, 128)],
                            rhs=q_rot[64:128, n0:n0 + NS],
                            start=True, stop=True, tile_position=(64, 0))
                        p_t = ptp.tile([128, 2 * NS], FP16, tag="p_t")
                        nc.scalar.activation(p_t[:], s_t[:], AF.Exp,
                                             scale=float(HD) ** -0.5)
                        pts[u] = p_t
                    if u >= LAG:
                        mv = u - LAG
                        nsv, mtv = divmod(mv, NMT)
                        nv0 = nsv * NS
                        if mtv == 8 and pending is not None:
                            emit_norm(pending)
                            pending = None
                        if p == 1 and nsv > 0 and NMT >= 32 and \
                                mtv in (12, 17, 22, 27):
                            proj_po(nv0 - NS, (mtv - 12) // 5, aux)
                        if mtv == 0:
                            oa_t = oap.tile([128, NS], F32, tag="oa")
                            ob_t = oap.tile([128, NS], F32, tag="ob")
                            accs[nsv] = (oa_t, ob_t)
                        oa, ob = accs[nsv]
                        p_t = pts.pop(mv)
                        nc.tensor.matmul(
                            oa[:], lhsT=v_sb[:, mtv, 2 * p + 0, :],
                            rhs=p_t[:, 0:NS],
                            start=(mtv == 0), stop=(mtv == NMT - 1))
                        nc.tensor.matmul(
                            ob[:], lhsT=v_sb[:, mtv, 2 * p + 1, :],
                            rhs=p_t[:, NS:2 * NS],
                            start=(mtv == 0), stop=(mtv == NMT - 1))
                        if mtv == NMT - 1:
                            ocs = []
                            for acc in accs.pop(nsv):
                                oc = ocp.tile([128, NS], F32, tag="oc")
                                nc.vector.tensor_copy(oc[:], acc[:])
                                ocs.append(oc)
                            if pending is not None:
                                emit_norm(pending)
                                pending = None
                            pending = (ocs, nv0)
                if pending is not None:
                    emit_norm(pending)
                if p == 1:
                    if NMT >= 32:
                        proj_chunk((NNS - 1) * NS, aux)
                    else:
                        for ns in range(NNS):
                            proj_chunk(ns * NS, aux)

            # one static psum split for the whole kernel: attention gets 6
            # banks (sp 4 + oap 2), everything else (qkv projection of BOTH
            # pairs, V build, output projection) shares the 2-bank aux pool.
            # This lets attention chunk 0 start while phase 1 is still
            # streaming (no bank-reuse serialization between phases).
            with (
                tc.tile_pool(name="sp", bufs=2, space="PSUM") as sp,
                tc.tile_pool(name="oap", bufs=1, space="PSUM") as oap,
                tc.tile_pool(name="aux", bufs=2, space="PSUM") as aux,
            ):
                q0, k0 = phase1(0, aux, aux)
                nc.sync.dma_start(
                    out=wp_sb[:],
                    in_=wp_d.rearrange("(ct p) m -> p ct m", p=128))
                phase3(0, q0, k0, sp, oap, aux)
                q1, k1 = phase1(1, aux, None)
                phase3(1, q1, k1, sp, oap, aux)

    nc.compile()
    return nc


def rope_tables(h, w, n):
    """cos/sin lookup tables, tiled x4 along partitions -> [128, n]."""
    quarter = HD // 4  # 16
    pos_h, pos_w = np.meshgrid(np.arange(h, dtype=np.float64),
                               np.arange(w, dtype=np.float64), indexing="ij")
    pos = np.stack([pos_h.ravel(), pos_w.ravel()], axis=-1)[:n]
    freqs = 1.0 / (MAX_FREQ ** (np.arange(quarter, dtype=np.float64) / quarter))
    ang = np.concatenate([pos[:, 0:1] * freqs, pos[:, 1:2] * freqs], axis=-1)
    cos = np.cos(ang).T.astype(np.float32)  # [32, n]
    sin = np.sin(ang).T.astype(np.float32)
    return np.tile(cos, (4, 1)), np.tile(sin, (4, 1))


def host_prep(x, w_qkv, w_proj, n=4096, h=H, w=W):
    """Build the 8 per-core input maps."""
    x = np.asarray(x, dtype=np.float32)
    w_qkv = np.asarray(w_qkv, dtype=np.float32)
    w_proj = np.asarray(w_proj, dtype=np.float32)
    dim = x.shape[1]
    cos128, sin128 = rope_tables(h, w, n)

    def jmat(wh):  # wh [64, dim] -> J @ wh
        return np.concatenate([-wh[32:64], wh[0:32]], axis=0)

    in_maps = []
    for c in range(N_CORES):
        b, g = c // 2, c % 2
        hs = [4 * g + i for i in range(4)]
        cols = []
        for pair in range(2):
            h0, h1 = hs[2 * pair], hs[2 * pair + 1]
            wq0, wq1 = w_qkv[64 * h0:64 * h0 + 64], w_qkv[64 * h1:64 * h1 + 64]
            wk0 = w_qkv[dim + 64 * h0: dim + 64 * h0 + 64]
            wk1 = w_qkv[dim + 64 * h1: dim + 64 * h1 + 64]
            cols += [wq0, wq1, jmat(wq0), jmat(wq1),
                     wk0, wk1, jmat(wk0), jmat(wk1),
                     np.zeros((128, dim), np.float32)]  # v slot unused
        wqkvT = np.concatenate(cols, axis=0).T.copy()  # [dim, 1280]

        wvT = np.zeros((dim, 256), np.float32)
        for i, hh in enumerate(hs):
            wvT[:, 64 * i:64 * i + 64] = w_qkv[2 * dim + 64 * hh:
                                               2 * dim + 64 * hh + 64].T
        wprojT = w_proj[:, 256 * g:256 * g + 256].T.copy()  # [256, dim]

        in_maps.append({
            "x": np.ascontiguousarray(x[b].reshape(dim, n)).astype(np.float16),
            "wqkvT": np.ascontiguousarray(wqkvT).astype(np.float16),
            "wvT": wvT.astype(np.float16),
            "wprojT": np.ascontiguousarray(wprojT).astype(np.float16),
            "cos": cos128[:, :n].copy(),
            "sin": sin128[:, :n].copy(),
        })
    return in_maps


_NC_CACHE = {}


def kernel(x, w_qkv, w_proj, trace=False):
    key = "full"
    if key not in _NC_CACHE:
        _NC_CACHE[key] = build_nc(**FULL)
    nc = _NC_CACHE[key]
    in_maps = host_prep(x, w_qkv, w_proj)
    res = run_bass_kernel_spmd(nc, in_maps, list(range(N_CORES)), trace=trace)
    outs = [res.results[c]["out"] for c in range(N_CORES)]
    full = np.empty((B, DIM, H, W), np.float32)
    for b in range(B):
        full[b] = (outs[2 * b] + outs[2 * b + 1]).reshape(DIM, H, W)
    kernel.last_results = res
    return full



# revision 7
# speedup vs baseline: 1.1897x; 1.1897x over previous
"""Multi-head self-attention (2D RoPE) Trainium2 Bass kernel.

Problem: x[4,512,64,64], w_qkv[1536,512], w_proj[512,512], 8 heads, hd=64,
N=4096 positions.  out = proj(attn(rope(q), rope(k)) @ v).

Sharding (8 cores): core c -> batch b=c//2, head-group g=c%2 (heads 4g..4g+3).
Each core computes a partial projection output [512, 4096] over its 256
attention-output channels; host sums the two partials per batch (the
"all-reduce" of the tensor-parallel split) and reshapes.

Per-core kernel design:
 - QKV projection as matmul with host-transposed weights; the RoPE rotation is
   linear in q, so the rotated-pair permutation J is folded into extra weight
   columns (Jq = J@Wq, Jk = J@Wk) and RoPE becomes q*COS + (Jq x)*SIN -- three
   full-width [128, n] vector ops, no per-head slicing.
 - Attention in transposed layout: S^T[m,n] = k_m . q_n via lhsT=k^T (64
   contract rows), two heads packed in the PE array via tile_position rows
   (0,0)/(64,0).  exp() runs on the scalar engine straight out of PSUM with the
   1/8 scale folded in; no max-subtraction (|S|/8 <= ~10, fp32 exp is safe).
 - P^T needs no transpose for the PV matmul (contract dim m is already on
   partitions), and V is produced directly in [m, d] layout by an extra
   matmul X^T @ Wv^T.  The PV stationary is [ones | zeros | V] (128 cols) so the matmul emits
   softmax denominators at psum partition 0 (where the fast reciprocal and
   partition_broadcast want them) and values at partitions 64-127.
 - The whole datapath runs fp16 (inputs cast on host): fp16 streams the PE
   at full 2.4 GHz where fp32/fp32r run at half rate, and all accumulation
   stays fp32 in PSUM, so the end-to-end error remains ~1e-3.
 - q/k tiles are double-buffered across the two head pairs so pair 1's
   projection overlaps pair 0's attention (it gets a 2-bank psum aux pool
   disjoint from attention's 6 banks); the output projection runs inside
   pair 1's attention loop per 512-column chunk.
"""

import numpy as np

import concourse.bass as bass
import concourse.mybir as mybir
import concourse.tile as tile
from concourse import bacc
from concourse.bass import ts
from concourse.bass_utils import run_bass_kernel_spmd

# ---- custom DVE ops: polynomial exp so the Vector engine can take a slice
# of the softmax exp off the saturated Scalar engine ----------------------
import concourse.dve_ops as dve_ops_mod
from concourse.dve_ops import DveOp
from concourse.dve_spec import (
    Spec, Src0, C0, C1, C2, C3, One, sq, lower as spec_lower,
    _spill_c3_to_src1, _has_src1,
)
from concourse.dve_uop import DveOpSpec

# exp(16*y) = poly4(y)^16 for y = S/128 (the 1/128 is folded into the k
# projection weights host-side).  poly4 is a c0=1-constrained minimax fit of
# exp on [-0.5, 0.5]; full-path rel err (incl. fp16 out) <= 9e-4 for |S/8|<=8.
EXP_C1 = 0.99984654
EXP_C2 = 0.50009464
EXP_C3 = 0.16931356
EXP_C4 = 0.04158808


def _register_dve_op(name, spec, subdim=False):
    for op in dve_ops_mod.OPS:
        if op.name == name:
            return op
    row = dve_ops_mod._CUSTOM_DVE_ROW_BASE + len(dve_ops_mod.OPS)
    assert row < 0x20, "custom DVE row field overflow"
    sha = {}
    for ver in ("v3", "v4"):
        s = DveOpSpec(name=name, opcode=row, uops=spec_lower(spec, ver=ver),
                      rd1_en=_has_src1(spec))
        sha[ver] = s.sha(ver)
    op = DveOp(name, spec, subdim=subdim, uops_sha=sha)
    dve_ops_mod.OPS.append(op)
    dve_ops_mod.CUSTOM_DVE_SPECS[name] = spec
    dve_ops_mod._SUB_OPCODE_FOR_NAME[name] = row
    return op


def _exp_p4_ref(in0, in1=None, s0=0.0, s1=0.0, imm2=0.0, *a, **k):
    c4 = in1 if in1 is not None else EXP_C4
    return 1.0 + in0 * (s0 + in0 * (s1 + in0 * (imm2 + in0 * c4)))


def _pow16_ref(in0, *a, **k):
    r = in0
    for _ in range(4):
        r = r * r
    return r


_y = Src0
EXP_P4_OP = _register_dve_op(
    "EXP_P4_ANT",
    Spec(body=_spill_c3_to_src1(
            One + _y * (C0 + _y * (C1 + _y * (C2 + _y * C3)))),
         reference=_exp_p4_ref))
POW16_OP = _register_dve_op(
    "POW16_ANT",
    Spec(body=sq(sq(sq(sq(Src0)))), reference=_pow16_ref))

F32 = mybir.dt.float32
F32R = mybir.dt.float32r
BF16 = mybir.dt.bfloat16
FP16 = mybir.dt.float16
AF = mybir.ActivationFunctionType

B, DIM, H, W = 4, 512, 64, 64
HEADS = 8
HD = 64
MAX_FREQ = 10000.0
N_CORES = 8

FULL = dict(N=4096, CH=512, NS=512)


def r(ap):
    return ap.bitcast(F32R)


def build_nc(N=4096, CH=512, NS=512):
    """Build the per-core Bass program (identical on all 8 cores)."""
    NMT = N // 128        # m tiles
    NCH = N // CH         # phase-1 chunks
    NNS = N // NS         # phase-3 chunks per head
    KC = DIM // 128       # contract tiles for qkv proj

    nc = bacc.Bacc("TRN2", target_bir_lowering=False, debug=False,
                   num_devices=N_CORES)

    x_d = nc.dram_tensor("x", [DIM, N], FP16, kind="ExternalInput").ap()
    wqkv_d = nc.dram_tensor("wqkvT", [DIM, 1280], FP16, kind="ExternalInput").ap()
    wv_d = nc.dram_tensor("wvT", [DIM, 256], FP16, kind="ExternalInput").ap()
    wp_d = nc.dram_tensor("wprojT", [256, DIM], FP16, kind="ExternalInput").ap()
    cos_d = nc.dram_tensor("cos", [128, N], F32, kind="ExternalInput").ap()
    sin_d = nc.dram_tensor("sin", [128, N], F32, kind="ExternalInput").ap()
    out_d = nc.dram_tensor("out", [DIM, N], F32, kind="ExternalOutput").ap()

    with tile.TileContext(nc) as tc:
        with (
            tc.tile_pool(name="singles", bufs=1) as singles,
            tc.tile_pool(name="qkpool", bufs=2) as qkpool,
            tc.tile_pool(name="xp", bufs=2) as xp,
            tc.tile_pool(name="csp", bufs=2) as csp,
            tc.tile_pool(name="ropep", bufs=2) as ropep,
            tc.tile_pool(name="ptp", bufs=10) as ptp,
            tc.tile_pool(name="pmp", bufs=2) as pmp,
            tc.tile_pool(name="nsm", bufs=6) as nsm,
            tc.tile_pool(name="ocp", bufs=8) as ocp,
            tc.tile_pool(name="osb", bufs=2) as osb,
        ):
            c4_sb = singles.tile([128, 1], F32, tag="c4")
            nc.vector.memset(c4_sb[:], EXP_C4)

            wq_sb = singles.tile([128, KC, 1280], FP16, tag="wq")
            for kc in range(KC):
                nc.sync.dma_start(
                    out=wq_sb[:, kc, :],
                    in_=wqkv_d[ts(kc, 128), :])
            wv_sb = singles.tile([128, KC, 256], FP16, tag="wv")
            nc.sync.dma_start(
                out=wv_sb[:],
                in_=wv_d.rearrange("(kc p) m -> p kc m", p=128))
            wp_sb = singles.tile([128, 2, DIM], FP16, tag="wp")

            v_sb = singles.tile([128, NMT, 4, 128], FP16, tag="v_sb")
            # pad lanes 1..63 of every head block are never written by the
            # compact V build; zero them once so the PV matmul streams zeros
            nc.vector.memset(v_sb[:, :, :, 0:64], 0.0)
            # one output tile PER CHUNK: a single big outT tile would create
            # whole-tile WAR deps (chunk c's normalize-write waiting chunk
            # c-1's projection-read), cascading every normalize one chunk late
            outs = [singles.tile([128, 2, NS], FP16, tag=f"outc{c}",
                                  name=f"outc{c}")
                    for c in range(NNS)]

            def phase1(p, pool, vpool):
                q_rot = qkpool.tile([128, N], FP16, tag="q_rot")
                k_rot = qkpool.tile([128, N], FP16, tag="k_rot")
                for ci in range(NCH):
                    c0 = ci * CH
                    x_t = xp.tile([128, KC, CH], FP16, tag="x_t")
                    nc.sync.dma_start(
                        out=x_t[:],
                        in_=x_d[:, c0:c0 + CH].rearrange(
                            "(kc p) n -> p kc n", p=128))
                    cos_t = csp.tile([128, CH], F32, tag="cos_t")
                    nc.sync.dma_start(out=cos_t[:], in_=cos_d[:, c0:c0 + CH])
                    sin_t = csp.tile([128, CH], F32, tag="sin_t")
                    nc.sync.dma_start(out=sin_t[:], in_=sin_d[:, c0:c0 + CH])

                    def mo_pair(mo_a, mo_b, dst):
                        ps_pair = []
                        for mo in (mo_a, mo_b):
                            ps = pool.tile([128, CH], F32, tag="aux")
                            col = p * 640 + mo * 128
                            for kc in range(KC):
                                nc.tensor.matmul(
                                    ps[:],
                                    lhsT=wq_sb[:, kc, col:col + 128],
                                    rhs=x_t[:, kc, :],
                                    start=(kc == 0), stop=(kc == KC - 1))
                            ps_pair.append(ps)
                        t1 = ropep.tile([128, CH], F32, tag="t1")
                        nc.vector.tensor_mul(t1[:], ps_pair[0][:], cos_t[:])
                        t2 = ropep.tile([128, CH], F32, tag="t2")
                        nc.vector.tensor_mul(t2[:], ps_pair[1][:], sin_t[:])
                        nc.vector.tensor_add(dst[:, c0:c0 + CH], t1[:], t2[:])

                    # k first: attention needs ALL k chunks (m dim) but only
                    # chunk ns of q, so k production is the critical path
                    mo_pair(2, 3, k_rot)
                    if p == 0:
                        # V (all 4 heads) in [m, d] layout: X^T @ Wv^T
                        for j in range(CH // 128):
                            mt = (c0 // 128) + j
                            vp = vpool.tile([128, 4, 64], F32, tag="aux")
                            for kc in range(KC):
                                nc.tensor.matmul(
                                    vp[:],
                                    lhsT=x_t[:, kc, ts(j, 128)],
                                    rhs=wv_sb[:, kc, :],
                                    start=(kc == 0), stop=(kc == KC - 1))
                            nc.vector.tensor_copy(
                                v_sb[:, mt, :, 64:128], vp[:])
                            nc.vector.memset(v_sb[:, mt, :, 0:1], 1.0)
                    mo_pair(0, 1, q_rot)
                return q_rot, k_rot

            def proj_po(n0, po, aux):
                pp = aux.tile([128, NS], F32, tag="aux")
                for ct in range(2):
                    nc.tensor.matmul(
                        pp[:],
                        lhsT=wp_sb[:, ct, ts(po, 128)],
                        rhs=outs[n0 // NS][:, ct, :],
                        start=(ct == 0), stop=(ct == 1))
                ot = osb.tile([128, NS], F32, tag="ot")
                nc.vector.tensor_copy(ot[:], pp[:])
                nc.sync.dma_start(out=out_d[ts(po, 128), n0:n0 + NS], in_=ot[:])

            def proj_chunk(n0, aux):
                # output projection for one finished 512-column chunk
                for po in range(4):
                    pp = aux.tile([128, NS], F32, tag="aux")
                    for ct in range(2):
                        nc.tensor.matmul(
                            pp[:],
                            lhsT=wp_sb[:, ct, ts(po, 128)],
                            rhs=outs[n0 // NS][:, ct, :],
                            start=(ct == 0), stop=(ct == 1))
                    ot = osb.tile([128, NS], F32, tag="ot")
                    nc.vector.tensor_copy(ot[:], pp[:])
                    nc.sync.dma_start(
                        out=out_d[ts(po, 128), n0:n0 + NS], in_=ot[:])

            def phase3(p, q_rot, k_rot, sp, oap, aux):
                LAG = min(5, NMT - 1)

                def emit_norm(pend):
                    ocs, pn0 = pend
                    for (oc, row0) in zip(ocs, (0, 64)):
                        rec = nsm.tile([1, NS], F32, tag="rec")
                        nc.vector.reciprocal_approx_fast(rec[:], oc[0:1, :])
                        rb = nsm.tile([128, NS], F32, tag="rb")
                        nc.gpsimd.partition_broadcast(rb[:], rec[:])
                        nc.vector.tensor_mul(
                            outs[pn0 // NS][row0:row0 + 64, p, :],
                            oc[64:128, :], rb[64:128, :])

                # one flat software pipeline over all (ns, mt) units: QK/exp
                # run LAG units ahead of PV *continuously across chunk
                # boundaries*, so the PE FIFO never drains its PV tail and the
                # scalar engine never starves at a boundary
                total = NNS * NMT
                pts = {}
                accs = {}
                pending = None
                for u in range(total + LAG):
                    if u < total:
                        ns, mt = divmod(u, NMT)
                        n0 = ns * NS
                        s_t = sp.tile([128, 2 * NS], F32, tag="s_t")
                        nc.tensor.matmul(
                            s_t[:, 0:NS],
                            lhsT=k_rot[0:64, ts(mt, 128)],
                            rhs=q_rot[0:64, n0:n0 + NS],
                            start=True, stop=True, tile_position=(0, 0))
                        nc.tensor.matmul(
                            s_t[:, NS:2 * NS],
                            lhsT=k_rot[64:128, ts(mt, 128)],
                            rhs=q_rot[64:128, n0:n0 + NS],
                            start=True, stop=True, tile_position=(64, 0))
                        p_t = ptp.tile([128, 2 * NS], FP16, tag="p_t")
                        if u % 5 == 3:
                            # vector-engine exp: poly4(S/128)^16; offloads
                            # ~20% of softmax exp from the scalar engine
                            pm = pmp.tile([128, 2 * NS], F32, tag="pm")
                            nc.vector._custom_dve(
                                EXP_P4_OP, out=pm[:], in0=s_t[:],
                                in1=c4_sb[:], s0=EXP_C1, s1=EXP_C2,
                                imm2=EXP_C3)
                            nc.vector._custom_dve(
                                POW16_OP, out=p_t[:], in0=pm[:])
                        else:
                            nc.scalar.activation(p_t[:], s_t[:], AF.Exp,
                                                 scale=16.0)
                        pts[u] = p_t
                    if u >= LAG:
                        mv = u - LAG
                        nsv, mtv = divmod(mv, NMT)
                        nv0 = nsv * NS
                        if mtv == 8 and pending is not None:
                            emit_norm(pending)
                            pending = None
                        if p == 1 and nsv > 0 and NMT >= 32 and \
                                mtv in (12, 17, 22, 27):
                            proj_po(nv0 - NS, (mtv - 12) // 5, aux)
                        if mtv == 0:
                            oa_t = oap.tile([128, NS], F32, tag="oa")
                            ob_t = oap.tile([128, NS], F32, tag="ob")
                            accs[nsv] = (oa_t, ob_t)
                        oa, ob = accs[nsv]
                        p_t = pts.pop(mv)
                        nc.tensor.matmul(
                            oa[:], lhsT=v_sb[:, mtv, 2 * p + 0, :],
                            rhs=p_t[:, 0:NS],
                            start=(mtv == 0), stop=(mtv == NMT - 1))
                        nc.tensor.matmul(
                            ob[:], lhsT=v_sb[:, mtv, 2 * p + 1, :],
                            rhs=p_t[:, NS:2 * NS],
                            start=(mtv == 0), stop=(mtv == NMT - 1))
                        if mtv == NMT - 1:
                            ocs = []
                            for acc in accs.pop(nsv):
                                oc = ocp.tile([128, NS], F32, tag="oc")
                                nc.vector.tensor_copy(oc[:], acc[:])
                                ocs.append(oc)
                            if pending is not None:
                                emit_norm(pending)
                                pending = None
                            pending = (ocs, nv0)
                if pending is not None:
                    emit_norm(pending)
                if p == 1:
                    if NMT >= 32:
                        proj_chunk((NNS - 1) * NS, aux)
                    else:
                        for ns in range(NNS):
                            proj_chunk(ns * NS, aux)

            # one static psum split for the whole kernel: attention gets 6
            # banks (sp 4 + oap 2), everything else (qkv projection of BOTH
            # pairs, V build, output projection) shares the 2-bank aux pool.
            # This lets attention chunk 0 start while phase 1 is still
            # streaming (no bank-reuse serialization between phases).
            with (
                tc.tile_pool(name="sp", bufs=2, space="PSUM") as sp,
                tc.tile_pool(name="oap", bufs=1, space="PSUM") as oap,
                tc.tile_pool(name="aux", bufs=2, space="PSUM") as aux,
            ):
                q0, k0 = phase1(0, aux, aux)
                nc.sync.dma_start(
                    out=wp_sb[:],
                    in_=wp_d.rearrange("(ct p) m -> p ct m", p=128))
                phase3(0, q0, k0, sp, oap, aux)
                q1, k1 = phase1(1, aux, None)
                phase3(1, q1, k1, sp, oap, aux)

    nc.compile()
    return nc


def rope_tables(h, w, n):
    """cos/sin lookup tables, tiled x4 along partitions -> [128, n]."""
    quarter = HD // 4  # 16
    pos_h, pos_w = np.meshgrid(np.arange(h, dtype=np.float64),
                               np.arange(w, dtype=np.float64), indexing="ij")
    pos = np.stack([pos_h.ravel(), pos_w.ravel()], axis=-1)[:n]
    freqs = 1.0 / (MAX_FREQ ** (np.arange(quarter, dtype=np.float64) / quarter))
    ang = np.concatenate([pos[:, 0:1] * freqs, pos[:, 1:2] * freqs], axis=-1)
    cos = np.cos(ang).T.astype(np.float32)  # [32, n]
    sin = np.sin(ang).T.astype(np.float32)
    return np.tile(cos, (4, 1)), np.tile(sin, (4, 1))


def host_prep(x, w_qkv, w_proj, n=4096, h=H, w=W):
    """Build the 8 per-core input maps."""
    x = np.asarray(x, dtype=np.float32)
    w_qkv = np.asarray(w_qkv, dtype=np.float32)
    w_proj = np.asarray(w_proj, dtype=np.float32)
    dim = x.shape[1]
    cos128, sin128 = rope_tables(h, w, n)

    def jmat(wh):  # wh [64, dim] -> J @ wh
        return np.concatenate([-wh[32:64], wh[0:32]], axis=0)

    in_maps = []
    for c in range(N_CORES):
        b, g = c // 2, c % 2
        hs = [4 * g + i for i in range(4)]
        cols = []
        for pair in range(2):
            h0, h1 = hs[2 * pair], hs[2 * pair + 1]
            wq0, wq1 = w_qkv[64 * h0:64 * h0 + 64], w_qkv[64 * h1:64 * h1 + 64]
            # 1/128 folded into k so scores arrive pre-scaled for the
            # poly-exp path (exp(S/8) = exp(16 * S/128))
            wk0 = w_qkv[dim + 64 * h0: dim + 64 * h0 + 64] / 128.0
            wk1 = w_qkv[dim + 64 * h1: dim + 64 * h1 + 64] / 128.0
            cols += [wq0, wq1, jmat(wq0), jmat(wq1),
                     wk0, wk1, jmat(wk0), jmat(wk1),
                     np.zeros((128, dim), np.float32)]  # v slot unused
        wqkvT = np.concatenate(cols, axis=0).T.copy()  # [dim, 1280]

        wvT = np.zeros((dim, 256), np.float32)
        for i, hh in enumerate(hs):
            wvT[:, 64 * i:64 * i + 64] = w_qkv[2 * dim + 64 * hh:
                                               2 * dim + 64 * hh + 64].T
        wprojT = w_proj[:, 256 * g:256 * g + 256].T.copy()  # [256, dim]

        in_maps.append({
            "x": np.ascontiguousarray(x[b].reshape(dim, n)).astype(np.float16),
            "wqkvT": np.ascontiguousarray(wqkvT).astype(np.float16),
            "wvT": wvT.astype(np.float16),
            "wprojT": np.ascontiguousarray(wprojT).astype(np.float16),
            "cos": cos128[:, :n].copy(),
            "sin": sin128[:, :n].copy(),
        })
    return in_maps


_NC_CACHE = {}


def kernel(x, w_qkv, w_proj, trace=False):
    key = "full"
    if key not in _NC_CACHE:
        _NC_CACHE[key] = build_nc(**FULL)
    nc = _NC_CACHE[key]
    in_maps = host_prep(x, w_qkv, w_proj)
    res = run_bass_kernel_spmd(nc, in_maps, list(range(N_CORES)), trace=trace)
    outs = [res.results[c]["out"] for c in range(N_CORES)]
    full = np.empty((B, DIM, H, W), np.float32)
    for b in range(B):
        full[b] = (outs[2 * b] + outs[2 * b + 1]).reshape(DIM, H, W)
    kernel.last_results = res
    return full



# revision 11
# speedup vs baseline: 1.2038x; 1.0118x over previous
"""Multi-head self-attention (2D RoPE) Trainium2 Bass kernel.

Problem: x[4,512,64,64], w_qkv[1536,512], w_proj[512,512], 8 heads, hd=64,
N=4096 positions.  out = proj(attn(rope(q), rope(k)) @ v).

Sharding (8 cores): core c -> batch b=c//2, head-group g=c%2 (heads 4g..4g+3).
Each core computes a partial projection output [512, 4096] over its 256
attention-output channels; host sums the two partials per batch (the
"all-reduce" of the tensor-parallel split) and reshapes.

Per-core kernel design:
 - QKV projection as matmul with host-transposed weights; the RoPE rotation is
   linear in q, so the rotated-pair permutation J is folded into extra weight
   columns (Jq = J@Wq, Jk = J@Wk) and RoPE becomes q*COS + (Jq x)*SIN -- three
   full-width [128, n] vector ops, no per-head slicing.
 - Attention in transposed layout: S^T[m,n] = k_m . q_n via lhsT=k^T (64
   contract rows), two heads packed in the PE array via tile_position rows
   (0,0)/(64,0).  exp() runs on the scalar engine straight out of PSUM with the
   1/8 scale folded in; no max-subtraction (|S|/8 <= ~10, fp32 exp is safe).
 - P^T needs no transpose for the PV matmul (contract dim m is already on
   partitions), and V is produced directly in [m, d] layout by an extra
   matmul X^T @ Wv^T.  The PV stationary is [ones | zeros | V] (128 cols) so the matmul emits
   softmax denominators at psum partition 0 (where the fast reciprocal and
   partition_broadcast want them) and values at partitions 64-127.
 - The whole datapath runs fp16 (inputs cast on host): fp16 streams the PE
   at full 2.4 GHz where fp32/fp32r run at half rate, and all accumulation
   stays fp32 in PSUM, so the end-to-end error remains ~1e-3.
 - q/k tiles are double-buffered across the two head pairs so pair 1's
   projection overlaps pair 0's attention (it gets a 2-bank psum aux pool
   disjoint from attention's 6 banks); the output projection runs inside
   pair 1's attention loop per 512-column chunk.
"""

import numpy as np

import concourse.bass as bass
import concourse.mybir as mybir
import concourse.tile as tile
from concourse import bacc
from concourse.bass import ts
from concourse.bass_utils import run_bass_kernel_spmd

# ---- custom DVE ops: polynomial exp so the Vector engine can take a slice
# of the softmax exp off the saturated Scalar engine ----------------------
import concourse.dve_ops as dve_ops_mod
from concourse.dve_ops import DveOp
from concourse.dve_spec import (
    Spec, Src0, C0, C1, C2, C3, One, sq, lower as spec_lower,
    _spill_c3_to_src1, _has_src1,
)
from concourse.dve_uop import DveOpSpec

# exp(16*y) = poly4(y)^16 for y = S/128 (the 1/128 is folded into the k
# projection weights host-side).  poly4 is a c0=1-constrained minimax fit of
# exp on [-0.5, 0.5]; full-path rel err (incl. fp16 out) <= 9e-4 for |S/8|<=8.
EXP_C1 = 0.99984654
EXP_C2 = 0.50009464
EXP_C3 = 0.16931356
EXP_C4 = 0.04158808


def _register_dve_op(name, spec, subdim=False, perf_en=False):
    for op in dve_ops_mod.OPS:
        if op.name == name:
            return op
    row = dve_ops_mod._CUSTOM_DVE_ROW_BASE + len(dve_ops_mod.OPS)
    assert row < 0x20, "custom DVE row field overflow"
    sha = {}
    for ver in ("v3", "v4"):
        s = DveOpSpec(name=name, opcode=row, uops=spec_lower(spec, ver=ver),
                      rd1_en=_has_src1(spec))
        sha[ver] = s.sha(ver)
    op = DveOp(name, spec, subdim=subdim, uops_sha=sha,
               perf_en={"v3": perf_en, "v4": perf_en})
    dve_ops_mod.OPS.append(op)
    dve_ops_mod.CUSTOM_DVE_SPECS[name] = spec
    dve_ops_mod._SUB_OPCODE_FOR_NAME[name] = row
    return op


def _exp_p4_ref(in0, in1=None, s0=0.0, s1=0.0, imm2=0.0, *a, **k):
    c4 = in1 if in1 is not None else EXP_C4
    return 1.0 + in0 * (s0 + in0 * (s1 + in0 * (imm2 + in0 * c4)))


def _pow16_ref(in0, *a, **k):
    r = in0
    for _ in range(4):
        r = r * r
    return r


_y = Src0
EXP_P4_OP = _register_dve_op(
    "EXP_P4_ANT",
    Spec(body=_spill_c3_to_src1(
            One + _y * (C0 + _y * (C1 + _y * (C2 + _y * C3)))),
         reference=_exp_p4_ref))
POW16_OP = _register_dve_op(
    "POW16_ANT",
    Spec(body=sq(sq(sq(sq(Src0)))), reference=_pow16_ref), perf_en=True)

F32 = mybir.dt.float32
F32R = mybir.dt.float32r
BF16 = mybir.dt.bfloat16
FP16 = mybir.dt.float16
AF = mybir.ActivationFunctionType

B, DIM, H, W = 4, 512, 64, 64
HEADS = 8
HD = 64
MAX_FREQ = 10000.0
N_CORES = 8

FULL = dict(N=4096, CH=512, NS=512)


def r(ap):
    return ap.bitcast(F32R)


def build_nc(N=4096, CH=512, NS=512):
    """Build the per-core Bass program (identical on all 8 cores)."""
    NMT = N // 128        # m tiles
    NCH = N // CH         # phase-1 chunks
    NNS = N // NS         # phase-3 chunks per head
    KC = DIM // 128       # contract tiles for qkv proj

    nc = bacc.Bacc("TRN2", target_bir_lowering=False, debug=False,
                   num_devices=N_CORES)

    x_d = nc.dram_tensor("x", [DIM, N], FP16, kind="ExternalInput").ap()
    wqkv_d = nc.dram_tensor("wqkvT", [DIM, 1280], FP16, kind="ExternalInput").ap()
    wv_d = nc.dram_tensor("wvT", [DIM, 256], FP16, kind="ExternalInput").ap()
    wp_d = nc.dram_tensor("wprojT", [256, DIM], FP16, kind="ExternalInput").ap()
    cos_d = nc.dram_tensor("cos", [128, N], F32, kind="ExternalInput").ap()
    sin_d = nc.dram_tensor("sin", [128, N], F32, kind="ExternalInput").ap()
    out_d = nc.dram_tensor("out", [DIM, N], F32, kind="ExternalOutput").ap()

    with tile.TileContext(nc) as tc:
        with (
            tc.tile_pool(name="singles", bufs=1) as singles,
            tc.tile_pool(name="qkpool", bufs=2) as qkpool,
            tc.tile_pool(name="xp", bufs=2) as xp,
            tc.tile_pool(name="csp", bufs=2) as csp,
            tc.tile_pool(name="ropep", bufs=2) as ropep,
            tc.tile_pool(name="ptp", bufs=10) as ptp,
            tc.tile_pool(name="pmp", bufs=2) as pmp,
            tc.tile_pool(name="nsm", bufs=6) as nsm,
            tc.tile_pool(name="ocp", bufs=8) as ocp,
            tc.tile_pool(name="osb", bufs=2) as osb,
        ):
            c4_sb = singles.tile([128, 1], F32, tag="c4")
            nc.vector.memset(c4_sb[:], EXP_C4)

            wq_sb = singles.tile([128, KC, 1280], FP16, tag="wq")
            for kc in range(KC):
                nc.sync.dma_start(
                    out=wq_sb[:, kc, :],
                    in_=wqkv_d[ts(kc, 128), :])
            wv_sb = singles.tile([128, KC, 256], FP16, tag="wv")
            nc.sync.dma_start(
                out=wv_sb[:],
                in_=wv_d.rearrange("(kc p) m -> p kc m", p=128))
            wp_sb = singles.tile([128, 2, DIM], FP16, tag="wp")

            v_sb = singles.tile([128, NMT, 4, 128], FP16, tag="v_sb")
            # pad lanes 1..63 of every head block are never written by the
            # compact V build; zero them once so the PV matmul streams zeros
            nc.vector.memset(v_sb[:, :, :, 0:64], 0.0)
            # one output tile PER CHUNK: a single big outT tile would create
            # whole-tile WAR deps (chunk c's normalize-write waiting chunk
            # c-1's projection-read), cascading every normalize one chunk late
            outs = [singles.tile([128, 2, NS], FP16, tag=f"outc{c}",
                                  name=f"outc{c}")
                    for c in range(NNS)]

            def phase1(p, pool, vpool):
                q_rot = qkpool.tile([128, N], FP16, tag="q_rot")
                k_rot = qkpool.tile([128, N], FP16, tag="k_rot")
                for ci in range(NCH):
                    c0 = ci * CH
                    x_t = xp.tile([128, KC, CH], FP16, tag="x_t")
                    nc.sync.dma_start(
                        out=x_t[:],
                        in_=x_d[:, c0:c0 + CH].rearrange(
                            "(kc p) n -> p kc n", p=128))
                    cos_t = csp.tile([128, CH], F32, tag="cos_t")
                    nc.sync.dma_start(out=cos_t[:], in_=cos_d[:, c0:c0 + CH])
                    sin_t = csp.tile([128, CH], F32, tag="sin_t")
                    nc.sync.dma_start(out=sin_t[:], in_=sin_d[:, c0:c0 + CH])

                    def mo_pair(mo_a, mo_b, dst):
                        ps_pair = []
                        for mo in (mo_a, mo_b):
                            ps = pool.tile([128, CH], F32, tag="aux")
                            col = p * 640 + mo * 128
                            for kc in range(KC):
                                nc.tensor.matmul(
                                    ps[:],
                                    lhsT=wq_sb[:, kc, col:col + 128],
                                    rhs=x_t[:, kc, :],
                                    start=(kc == 0), stop=(kc == KC - 1))
                            ps_pair.append(ps)
                        t1 = ropep.tile([128, CH], F32, tag="t1")
                        nc.vector.tensor_mul(t1[:], ps_pair[0][:], cos_t[:])
                        t2 = ropep.tile([128, CH], F32, tag="t2")
                        nc.vector.tensor_mul(t2[:], ps_pair[1][:], sin_t[:])
                        nc.vector.tensor_add(dst[:, c0:c0 + CH], t1[:], t2[:])

                    # k first: attention needs ALL k chunks (m dim) but only
                    # chunk ns of q, so k production is the critical path
                    mo_pair(2, 3, k_rot)
                    if p == 0:
                        # V (all 4 heads) in [m, d] layout: X^T @ Wv^T
                        for j in range(CH // 128):
                            mt = (c0 // 128) + j
                            vp = vpool.tile([128, 4, 64], F32, tag="aux")
                            for kc in range(KC):
                                nc.tensor.matmul(
                                    vp[:],
                                    lhsT=x_t[:, kc, ts(j, 128)],
                                    rhs=wv_sb[:, kc, :],
                                    start=(kc == 0), stop=(kc == KC - 1))
                            nc.vector.tensor_copy(
                                v_sb[:, mt, :, 64:128], vp[:])
                            nc.vector.memset(v_sb[:, mt, :, 0:1], 1.0)
                    mo_pair(0, 1, q_rot)
                return q_rot, k_rot

            def proj_po(n0, po, aux):
                pp = aux.tile([128, NS], F32, tag="aux")
                for ct in range(2):
                    nc.tensor.matmul(
                        pp[:],
                        lhsT=wp_sb[:, ct, ts(po, 128)],
                        rhs=outs[n0 // NS][:, ct, :],
                        start=(ct == 0), stop=(ct == 1))
                ot = osb.tile([128, NS], F32, tag="ot")
                nc.vector.tensor_copy(ot[:], pp[:])
                nc.sync.dma_start(out=out_d[ts(po, 128), n0:n0 + NS], in_=ot[:])

            def proj_chunk(n0, aux):
                # output projection for one finished 512-column chunk
                for po in range(4):
                    pp = aux.tile([128, NS], F32, tag="aux")
                    for ct in range(2):
                        nc.tensor.matmul(
                            pp[:],
                            lhsT=wp_sb[:, ct, ts(po, 128)],
                            rhs=outs[n0 // NS][:, ct, :],
                            start=(ct == 0), stop=(ct == 1))
                    ot = osb.tile([128, NS], F32, tag="ot")
                    nc.vector.tensor_copy(ot[:], pp[:])
                    nc.sync.dma_start(
                        out=out_d[ts(po, 128), n0:n0 + NS], in_=ot[:])

            def phase3(p, q_rot, k_rot, sp, oap, aux):
                LAG = min(5, NMT - 1)

                def emit_norm(pend):
                    ocs, pn0 = pend
                    for (oc, row0) in zip(ocs, (0, 64)):
                        rec = nsm.tile([1, NS], F32, tag="rec")
                        nc.vector.reciprocal_approx_fast(rec[:], oc[0:1, :])
                        rb = nsm.tile([128, NS], F32, tag="rb")
                        nc.gpsimd.partition_broadcast(rb[:], rec[:])
                        nc.vector.tensor_mul(
                            outs[pn0 // NS][row0:row0 + 64, p, :],
                            oc[64:128, :], rb[64:128, :])

                # one flat software pipeline over all (ns, mt) units: QK/exp
                # run LAG units ahead of PV *continuously across chunk
                # boundaries*, so the PE FIFO never drains its PV tail and the
                # scalar engine never starves at a boundary
                total = NNS * NMT
                pts = {}
                accs = {}
                pending = None
                for u in range(total + LAG):
                    if u < total:
                        ns, mt = divmod(u, NMT)
                        n0 = ns * NS
                        s_t = sp.tile([128, 2 * NS], F32, tag="s_t")
                        nc.tensor.matmul(
                            s_t[:, 0:NS],
                            lhsT=k_rot[0:64, ts(mt, 128)],
                            rhs=q_rot[0:64, n0:n0 + NS],
                            start=True, stop=True, tile_position=(0, 0))
                        nc.tensor.matmul(
                            s_t[:, NS:2 * NS],
                            lhsT=k_rot[64:128, ts(mt, 128)],
                            rhs=q_rot[64:128, n0:n0 + NS],
                            start=True, stop=True, tile_position=(64, 0))
                        p_t = ptp.tile([128, 2 * NS], FP16, tag="p_t")
                        if u % 5 == 3:
                            # vector-engine exp: poly4(S/128)^16; offloads
                            # ~20% of softmax exp from the scalar engine.
                            # high_priority keeps the pair at the DVE queue
                            # head so s_t frees before QK(u+2) needs it
                            pm = pmp.tile([128, 2 * NS], FP16, tag="pm")
                            with tc.high_priority():
                                nc.vector._custom_dve(
                                    EXP_P4_OP, out=pm[:], in0=s_t[:],
                                    in1=c4_sb[:], s0=EXP_C1, s1=EXP_C2,
                                    imm2=EXP_C3)
                                nc.vector._custom_dve(
                                    POW16_OP, out=p_t[:], in0=pm[:])
                        else:
                            nc.scalar.activation(p_t[:], s_t[:], AF.Exp,
                                                 scale=16.0)
                        pts[u] = p_t
                    if u >= LAG:
                        mv = u - LAG
                        nsv, mtv = divmod(mv, NMT)
                        nv0 = nsv * NS
                        if mtv == 8 and pending is not None:
                            emit_norm(pending)
                            pending = None
                        if p == 1 and nsv > 0 and NMT >= 32 and \
                                mtv in (12, 17, 22, 27):
                            proj_po(nv0 - NS, (mtv - 12) // 5, aux)
                        if mtv == 0:
                            oa_t = oap.tile([128, NS], F32, tag="oa")
                            ob_t = oap.tile([128, NS], F32, tag="ob")
                            accs[nsv] = (oa_t, ob_t)
                        oa, ob = accs[nsv]
                        p_t = pts.pop(mv)
                        nc.tensor.matmul(
                            oa[:], lhsT=v_sb[:, mtv, 2 * p + 0, :],
                            rhs=p_t[:, 0:NS],
                            start=(mtv == 0), stop=(mtv == NMT - 1))
                        nc.tensor.matmul(
                            ob[:], lhsT=v_sb[:, mtv, 2 * p + 1, :],
                            rhs=p_t[:, NS:2 * NS],
                            start=(mtv == 0), stop=(mtv == NMT - 1))
                        if mtv == NMT - 1:
                            ocs = []
                            for acc in accs.pop(nsv):
                                oc = ocp.tile([128, NS], F32, tag="oc")
                                # evacuate on the scalar engine: it has slack
                                # now that 1/5 of the exps moved to the DVE
                                nc.scalar.copy(oc[:], acc[:])
                                ocs.append(oc)
                            if pending is not None:
                                emit_norm(pending)
                                pending = None
                            pending = (ocs, nv0)
                if pending is not None:
                    emit_norm(pending)
                if p == 1:
                    if NMT >= 32:
                        proj_chunk((NNS - 1) * NS, aux)
                    else:
                        for ns in range(NNS):
                            proj_chunk(ns * NS, aux)

            # one static psum split for the whole kernel: attention gets 6
            # banks (sp 4 + oap 2), everything else (qkv projection of BOTH
            # pairs, V build, output projection) shares the 2-bank aux pool.
            # This lets attention chunk 0 start while phase 1 is still
            # streaming (no bank-reuse serialization between phases).
            with (
                tc.tile_pool(name="sp", bufs=2, space="PSUM") as sp,
                tc.tile_pool(name="oap", bufs=1, space="PSUM") as oap,
                tc.tile_pool(name="aux", bufs=2, space="PSUM") as aux,
            ):
                q0, k0 = phase1(0, aux, aux)
                nc.sync.dma_start(
                    out=wp_sb[:],
                    in_=wp_d.rearrange("(ct p) m -> p ct m", p=128))
                phase3(0, q0, k0, sp, oap, aux)
                q1, k1 = phase1(1, aux, None)
                phase3(1, q1, k1, sp, oap, aux)

    nc.compile()
    return nc


def rope_tables(h, w, n):
    """cos/sin lookup tables, tiled x4 along partitions -> [128, n]."""
    quarter = HD // 4  # 16
    pos_h, pos_w = np.meshgrid(np.arange(h, dtype=np.float64),
                               np.arange(w, dtype=np.float64), indexing="ij")
    pos = np.stack([pos_h.ravel(), pos_w.ravel()], axis=-1)[:n]
    freqs = 1.0 / (MAX_FREQ ** (np.arange(quarter, dtype=np.float64) / quarter))
    ang = np.concatenate([pos[:, 0:1] * freqs, pos[:, 1:2] * freqs], axis=-1)
    cos = np.cos(ang).T.astype(np.float32)  # [32, n]
    sin = np.sin(ang).T.astype(np.float32)
    return np.tile(cos, (4, 1)), np.tile(sin, (4, 1))


def host_prep(x, w_qkv, w_proj, n=4096, h=H, w=W):
    """Build the 8 per-core input maps."""
    x = np.asarray(x, dtype=np.float32)
    w_qkv = np.asarray(w_qkv, dtype=np.float32)
    w_proj = np.asarray(w_proj, dtype=np.float32)
    dim = x.shape[1]
    cos128, sin128 = rope_tables(h, w, n)

    def jmat(wh):  # wh [64, dim] -> J @ wh
        return np.concatenate([-wh[32:64], wh[0:32]], axis=0)

    in_maps = []
    for c in range(N_CORES):
        b, g = c // 2, c % 2
        hs = [4 * g + i for i in range(4)]
        cols = []
        for pair in range(2):
            h0, h1 = hs[2 * pair], hs[2 * pair + 1]
            wq0, wq1 = w_qkv[64 * h0:64 * h0 + 64], w_qkv[64 * h1:64 * h1 + 64]
            # 1/128 folded into k so scores arrive pre-scaled for the
            # poly-exp path (exp(S/8) = exp(16 * S/128))
            wk0 = w_qkv[dim + 64 * h0: dim + 64 * h0 + 64] / 128.0
            wk1 = w_qkv[dim + 64 * h1: dim + 64 * h1 + 64] / 128.0
            cols += [wq0, wq1, jmat(wq0), jmat(wq1),
                     wk0, wk1, jmat(wk0), jmat(wk1),
                     np.zeros((128, dim), np.float32)]  # v slot unused
        wqkvT = np.concatenate(cols, axis=0).T.copy()  # [dim, 1280]

        wvT = np.zeros((dim, 256), np.float32)
        for i, hh in enumerate(hs):
            wvT[:, 64 * i:64 * i + 64] = w_qkv[2 * dim + 64 * hh:
                                               2 * dim + 64 * hh + 64].T
        wprojT = w_proj[:, 256 * g:256 * g + 256].T.copy()  # [256, dim]

        in_maps.append({
            "x": np.ascontiguousarray(x[b].reshape(dim, n)).astype(np.float16),
            "wqkvT": np.ascontiguousarray(wqkvT).astype(np.float16),
            "wvT": wvT.astype(np.float16),
            "wprojT": np.ascontiguousarray(wprojT).astype(np.float16),
            "cos": cos128[:, :n].copy(),
            "sin": sin128[:, :n].copy(),
        })
    return in_maps


_NC_CACHE = {}


def kernel(x, w_qkv, w_proj, trace=False):
    key = "full"
    if key not in _NC_CACHE:
        _NC_CACHE[key] = build_nc(**FULL)
    nc = _NC_CACHE[key]
    in_maps = host_prep(x, w_qkv, w_proj)
    res = run_bass_kernel_spmd(nc, in_maps, list(range(N_CORES)), trace=trace)
    outs = [res.results[c]["out"] for c in range(N_CORES)]
    full = np.empty((B, DIM, H, W), np.float32)
    for b in range(B):
        full[b] = (outs[2 * b] + outs[2 * b + 1]).reshape(DIM, H, W)
    kernel.last_results = res
    return full



# revision 13
# speedup vs baseline: 1.2055x; 1.0014x over previous
"""Multi-head self-attention (2D RoPE) Trainium2 Bass kernel.

Problem: x[4,512,64,64], w_qkv[1536,512], w_proj[512,512], 8 heads, hd=64,
N=4096 positions.  out = proj(attn(rope(q), rope(k)) @ v).

Sharding (8 cores): core c -> batch b=c//2, head-group g=c%2 (heads 4g..4g+3).
Each core computes a partial projection output [512, 4096] over its 256
attention-output channels; host sums the two partials per batch (the
"all-reduce" of the tensor-parallel split) and reshapes.

Per-core kernel design:
 - QKV projection as matmul with host-transposed weights; the RoPE rotation is
   linear in q, so the rotated-pair permutation J is folded into extra weight
   columns (Jq = J@Wq, Jk = J@Wk) and RoPE becomes q*COS + (Jq x)*SIN -- three
   full-width [128, n] vector ops, no per-head slicing.
 - Attention in transposed layout: S^T[m,n] = k_m . q_n via lhsT=k^T (64
   contract rows), two heads packed in the PE array via tile_position rows
   (0,0)/(64,0).  exp() runs on the scalar engine straight out of PSUM with the
   1/8 scale folded in; no max-subtraction (|S|/8 <= ~10, fp32 exp is safe).
 - P^T needs no transpose for the PV matmul (contract dim m is already on
   partitions), and V is produced directly in [m, d] layout by an extra
   matmul X^T @ Wv^T.  The PV stationary is [ones | zeros | V] (128 cols) so the matmul emits
   softmax denominators at psum partition 0 (where the fast reciprocal and
   partition_broadcast want them) and values at partitions 64-127.
 - The whole datapath runs fp16 (inputs cast on host): fp16 streams the PE
   at full 2.4 GHz where fp32/fp32r run at half rate, and all accumulation
   stays fp32 in PSUM, so the end-to-end error remains ~1e-3.
 - q/k tiles are double-buffered across the two head pairs so pair 1's
   projection overlaps pair 0's attention (it gets a 2-bank psum aux pool
   disjoint from attention's 6 banks); the output projection runs inside
   pair 1's attention loop per 512-column chunk.
"""

import numpy as np

import concourse.bass as bass
import concourse.mybir as mybir
import concourse.tile as tile
from concourse import bacc
from concourse.bass import ts
from concourse.bass_utils import run_bass_kernel_spmd

# ---- custom DVE ops: polynomial exp so the Vector engine can take a slice
# of the softmax exp off the saturated Scalar engine ----------------------
import concourse.dve_ops as dve_ops_mod
from concourse.dve_ops import DveOp
from concourse.dve_spec import (
    Spec, Src0, C0, C1, C2, C3, One, sq, lower as spec_lower,
    _spill_c3_to_src1, _has_src1,
)
from concourse.dve_uop import DveOpSpec

# exp(16*y) = poly4(y)^16 for y = S/128 (the 1/128 is folded into the k
# projection weights host-side).  poly4 is a c0=1-constrained minimax fit of
# exp on [-0.5, 0.5]; full-path rel err (incl. fp16 out) <= 9e-4 for |S/8|<=8.
EXP_C1 = 0.99984654
EXP_C2 = 0.50009464
EXP_C3 = 0.16931356
EXP_C4 = 0.04158808


def _register_dve_op(name, spec, subdim=False, perf_en=False):
    for op in dve_ops_mod.OPS:
        if op.name == name:
            return op
    row = dve_ops_mod._CUSTOM_DVE_ROW_BASE + len(dve_ops_mod.OPS)
    assert row < 0x20, "custom DVE row field overflow"
    sha = {}
    for ver in ("v3", "v4"):
        s = DveOpSpec(name=name, opcode=row, uops=spec_lower(spec, ver=ver),
                      rd1_en=_has_src1(spec))
        sha[ver] = s.sha(ver)
    op = DveOp(name, spec, subdim=subdim, uops_sha=sha,
               perf_en={"v3": perf_en, "v4": perf_en})
    dve_ops_mod.OPS.append(op)
    dve_ops_mod.CUSTOM_DVE_SPECS[name] = spec
    dve_ops_mod._SUB_OPCODE_FOR_NAME[name] = row
    return op


def _exp_p4_ref(in0, in1=None, s0=0.0, s1=0.0, imm2=0.0, *a, **k):
    c4 = in1 if in1 is not None else EXP_C4
    return 1.0 + in0 * (s0 + in0 * (s1 + in0 * (imm2 + in0 * c4)))


def _pow16_ref(in0, *a, **k):
    r = in0
    for _ in range(4):
        r = r * r
    return r


_y = Src0
EXP_P4_OP = _register_dve_op(
    "EXP_P4_ANT",
    Spec(body=_spill_c3_to_src1(
            One + _y * (C0 + _y * (C1 + _y * (C2 + _y * C3)))),
         reference=_exp_p4_ref))
POW16_OP = _register_dve_op(
    "POW16_ANT",
    Spec(body=sq(sq(sq(sq(Src0)))), reference=_pow16_ref), perf_en=True)

F32 = mybir.dt.float32
F32R = mybir.dt.float32r
BF16 = mybir.dt.bfloat16
FP16 = mybir.dt.float16
AF = mybir.ActivationFunctionType

B, DIM, H, W = 4, 512, 64, 64
HEADS = 8
HD = 64
MAX_FREQ = 10000.0
N_CORES = 8

FULL = dict(N=4096, CH=512, NS=512)


def r(ap):
    return ap.bitcast(F32R)


def build_nc(N=4096, CH=512, NS=512):
    """Build the per-core Bass program (identical on all 8 cores)."""
    NMT = N // 128        # m tiles
    NCH = N // CH         # phase-1 chunks
    NNS = N // NS         # phase-3 chunks per head
    KC = DIM // 128       # contract tiles for qkv proj

    nc = bacc.Bacc("TRN2", target_bir_lowering=False, debug=False,
                   num_devices=N_CORES)

    x_d = nc.dram_tensor("x", [DIM, N], FP16, kind="ExternalInput").ap()
    wqkv_d = nc.dram_tensor("wqkvT", [DIM, 1280], FP16, kind="ExternalInput").ap()
    wv_d = nc.dram_tensor("wvT", [DIM, 256], FP16, kind="ExternalInput").ap()
    wp_d = nc.dram_tensor("wprojT", [256, DIM], FP16, kind="ExternalInput").ap()
    cos_d = nc.dram_tensor("cos", [128, N], F32, kind="ExternalInput").ap()
    sin_d = nc.dram_tensor("sin", [128, N], F32, kind="ExternalInput").ap()
    out_d = nc.dram_tensor("out", [DIM, N], F32, kind="ExternalOutput").ap()

    with tile.TileContext(nc) as tc:
        with (
            tc.tile_pool(name="singles", bufs=1) as singles,
            tc.tile_pool(name="qkpool", bufs=2) as qkpool,
            tc.tile_pool(name="xp", bufs=2) as xp,
            tc.tile_pool(name="csp", bufs=2) as csp,
            tc.tile_pool(name="ropep", bufs=2) as ropep,
            tc.tile_pool(name="ptp", bufs=10) as ptp,
            tc.tile_pool(name="pmp", bufs=2) as pmp,
            tc.tile_pool(name="nsm", bufs=6) as nsm,
            tc.tile_pool(name="ocp", bufs=8) as ocp,
            tc.tile_pool(name="osb", bufs=2) as osb,
        ):
            c4_sb = singles.tile([128, 1], F32, tag="c4")
            nc.vector.memset(c4_sb[:], EXP_C4)

            wq_sb = singles.tile([128, KC, 1280], FP16, tag="wq")
            for kc in range(KC):
                nc.sync.dma_start(
                    out=wq_sb[:, kc, :],
                    in_=wqkv_d[ts(kc, 128), :])
            wv_sb = singles.tile([128, KC, 256], FP16, tag="wv")
            nc.sync.dma_start(
                out=wv_sb[:],
                in_=wv_d.rearrange("(kc p) m -> p kc m", p=128))
            wp_sb = singles.tile([128, 2, DIM], FP16, tag="wp")

            v_sb = singles.tile([128, NMT, 4, 128], FP16, tag="v_sb")
            # pad lanes 1..63 of every head block are never written by the
            # compact V build; zero them once so the PV matmul streams zeros
            nc.vector.memset(v_sb[:, :, :, 0:64], 0.0)
            # one output tile PER CHUNK: a single big outT tile would create
            # whole-tile WAR deps (chunk c's normalize-write waiting chunk
            # c-1's projection-read), cascading every normalize one chunk late
            outs = [singles.tile([128, 2, NS], FP16, tag=f"outc{c}",
                                  name=f"outc{c}")
                    for c in range(NNS)]

            def phase1(p, pool, vpool):
                q_rot = qkpool.tile([128, N], FP16, tag="q_rot")
                k_rot = qkpool.tile([128, N], FP16, tag="k_rot")
                for ci in range(NCH):
                    c0 = ci * CH
                    x_t = xp.tile([128, KC, CH], FP16, tag="x_t")
                    nc.sync.dma_start(
                        out=x_t[:],
                        in_=x_d[:, c0:c0 + CH].rearrange(
                            "(kc p) n -> p kc n", p=128))
                    cos_t = csp.tile([128, CH], F32, tag="cos_t")
                    nc.sync.dma_start(out=cos_t[:], in_=cos_d[:, c0:c0 + CH])
                    sin_t = csp.tile([128, CH], F32, tag="sin_t")
                    nc.sync.dma_start(out=sin_t[:], in_=sin_d[:, c0:c0 + CH])

                    def mo_pair(mo_a, mo_b, dst):
                        ps_pair = []
                        for mo in (mo_a, mo_b):
                            ps = pool.tile([128, CH], F32, tag="aux")
                            col = p * 640 + mo * 128
                            for kc in range(KC):
                                nc.tensor.matmul(
                                    ps[:],
                                    lhsT=wq_sb[:, kc, col:col + 128],
                                    rhs=x_t[:, kc, :],
                                    start=(kc == 0), stop=(kc == KC - 1))
                            ps_pair.append(ps)
                        t1 = ropep.tile([128, CH], F32, tag="t1")
                        nc.vector.tensor_mul(t1[:], ps_pair[0][:], cos_t[:])
                        t2 = ropep.tile([128, CH], F32, tag="t2")
                        nc.vector.tensor_mul(t2[:], ps_pair[1][:], sin_t[:])
                        nc.vector.tensor_add(dst[:, c0:c0 + CH], t1[:], t2[:])

                    # k first: attention needs ALL k chunks (m dim) but only
                    # chunk ns of q, so k production is the critical path
                    mo_pair(2, 3, k_rot)
                    if p == 0:
                        # V (all 4 heads) in [m, d] layout: X^T @ Wv^T
                        for j in range(CH // 128):
                            mt = (c0 // 128) + j
                            vp = vpool.tile([128, 4, 64], F32, tag="aux")
                            for kc in range(KC):
                                nc.tensor.matmul(
                                    vp[:],
                                    lhsT=x_t[:, kc, ts(j, 128)],
                                    rhs=wv_sb[:, kc, :],
                                    start=(kc == 0), stop=(kc == KC - 1))
                            nc.vector.tensor_copy(
                                v_sb[:, mt, :, 64:128], vp[:])
                            nc.vector.memset(v_sb[:, mt, :, 0:1], 1.0)
                    mo_pair(0, 1, q_rot)
                return q_rot, k_rot

            def proj_po(n0, po, aux):
                pp = aux.tile([128, NS], F32, tag="aux")
                for ct in range(2):
                    nc.tensor.matmul(
                        pp[:],
                        lhsT=wp_sb[:, ct, ts(po, 128)],
                        rhs=outs[n0 // NS][:, ct, :],
                        start=(ct == 0), stop=(ct == 1))
                ot = osb.tile([128, NS], F32, tag="ot")
                nc.vector.tensor_copy(ot[:], pp[:])
                nc.sync.dma_start(out=out_d[ts(po, 128), n0:n0 + NS], in_=ot[:])

            def proj_chunk(n0, aux):
                # output projection for one finished 512-column chunk
                for po in range(4):
                    pp = aux.tile([128, NS], F32, tag="aux")
                    for ct in range(2):
                        nc.tensor.matmul(
                            pp[:],
                            lhsT=wp_sb[:, ct, ts(po, 128)],
                            rhs=outs[n0 // NS][:, ct, :],
                            start=(ct == 0), stop=(ct == 1))
                    ot = osb.tile([128, NS], F32, tag="ot")
                    nc.vector.tensor_copy(ot[:], pp[:])
                    nc.sync.dma_start(
                        out=out_d[ts(po, 128), n0:n0 + NS], in_=ot[:])

            def phase3(p, q_rot, k_rot, sp, oap, aux):
                LAG = 6 if NMT > 6 else 2

                def emit_norm(pend):
                    ocs, pn0 = pend
                    for (oc, row0) in zip(ocs, (0, 64)):
                        rec = nsm.tile([1, NS], F32, tag="rec")
                        nc.vector.reciprocal_approx_fast(rec[:], oc[0:1, :])
                        rb = nsm.tile([128, NS], F32, tag="rb")
                        nc.gpsimd.partition_broadcast(rb[:], rec[:])
                        nc.vector.tensor_mul(
                            outs[pn0 // NS][row0:row0 + 64, p, :],
                            oc[64:128, :], rb[64:128, :])

                # one flat software pipeline over all (ns, mt) units: QK/exp
                # run LAG units ahead of PV *continuously across chunk
                # boundaries*.  Units are processed in PAIRS: the 4 QK
                # matmuls of two adjacent units are emitted back-to-back so
                # their 64-row tile_position groups overlap in the PE array
                # and the stationary-swap (row-conflict drain) stalls are
                # paid once per pair instead of once per unit.
                total = NNS * NMT
                pts = {}
                sts = {}
                accs = {}
                pending = None

                def emit_qk(u):
                    ns, mt = divmod(u, NMT)
                    n0 = ns * NS
                    s_t = sp.tile([128, 2 * NS], F32, tag="s_t")
                    nc.tensor.matmul(
                        s_t[:, 0:NS],
                        lhsT=k_rot[0:64, ts(mt, 128)],
                        rhs=q_rot[0:64, n0:n0 + NS],
                        start=True, stop=True, tile_position=(0, 0))
                    nc.tensor.matmul(
                        s_t[:, NS:2 * NS],
                        lhsT=k_rot[64:128, ts(mt, 128)],
                        rhs=q_rot[64:128, n0:n0 + NS],
                        start=True, stop=True, tile_position=(64, 0))
                    sts[u] = s_t

                def emit_exp(u):
                    s_t = sts.pop(u)
                    p_t = ptp.tile([128, 2 * NS], FP16, tag="p_t")
                    if u % 5 == 3:
                        # vector-engine exp: poly4(S/128)^16; offloads ~20%
                        # of softmax exp from the scalar engine.
                        # high_priority keeps the pair at the DVE queue head
                        # so s_t frees before the next pair's QK needs it
                        pm = pmp.tile([128, 2 * NS], FP16, tag="pm")
                        with tc.high_priority():
                            nc.vector._custom_dve(
                                EXP_P4_OP, out=pm[:], in0=s_t[:],
                                in1=c4_sb[:], s0=EXP_C1, s1=EXP_C2,
                                imm2=EXP_C3)
                            nc.vector._custom_dve(
                                POW16_OP, out=p_t[:], in0=pm[:])
                    else:
                        nc.scalar.activation(p_t[:], s_t[:], AF.Exp,
                                             scale=16.0)
                    pts[u] = p_t

                def emit_pv(mv):
                    nonlocal pending
                    nsv, mtv = divmod(mv, NMT)
                    nv0 = nsv * NS
                    if mtv == 8 and pending is not None:
                        emit_norm(pending)
                        pending = None
                    if p == 1 and nsv > 0 and NMT >= 32 and \
                            mtv in (12, 17, 22, 27):
                        proj_po(nv0 - NS, (mtv - 12) // 5, aux)
                    if mtv == 0:
                        oa_t = oap.tile([128, NS], F32, tag="oa")
                        ob_t = oap.tile([128, NS], F32, tag="ob")
                        accs[nsv] = (oa_t, ob_t)
                    oa, ob = accs[nsv]
                    p_t = pts.pop(mv)
                    nc.tensor.matmul(
                        oa[:], lhsT=v_sb[:, mtv, 2 * p + 0, :],
                        rhs=p_t[:, 0:NS],
                        start=(mtv == 0), stop=(mtv == NMT - 1))
                    nc.tensor.matmul(
                        ob[:], lhsT=v_sb[:, mtv, 2 * p + 1, :],
                        rhs=p_t[:, NS:2 * NS],
                        start=(mtv == 0), stop=(mtv == NMT - 1))
                    if mtv == NMT - 1:
                        ocs = []
                        for acc in accs.pop(nsv):
                            oc = ocp.tile([128, NS], F32, tag="oc")
                            # evacuate on the scalar engine: it has slack
                            # now that 1/5 of the exps moved to the DVE
                            nc.scalar.copy(oc[:], acc[:])
                            ocs.append(oc)
                        if pending is not None:
                            emit_norm(pending)
                            pending = None
                        pending = (ocs, nv0)

                for base in range(0, total + LAG, 2):
                    for u in (base, base + 1):
                        if u < total:
                            emit_qk(u)
                    for u in (base, base + 1):
                        if u < total:
                            emit_exp(u)
                    for u in (base, base + 1):
                        mv = u - LAG
                        if 0 <= mv < total:
                            emit_pv(mv)
                if pending is not None:
                    emit_norm(pending)
                if p == 1:
                    if NMT >= 32:
                        proj_chunk((NNS - 1) * NS, aux)
                    else:
                        for ns in range(NNS):
                            proj_chunk(ns * NS, aux)

            # one static psum split for the whole kernel: attention gets 6
            # banks (sp 4 + oap 2), everything else (qkv projection of BOTH
            # pairs, V build, output projection) shares the 2-bank aux pool.
            # This lets attention chunk 0 start while phase 1 is still
            # streaming (no bank-reuse serialization between phases).
            with (
                tc.tile_pool(name="sp", bufs=2, space="PSUM") as sp,
                tc.tile_pool(name="oap", bufs=1, space="PSUM") as oap,
                tc.tile_pool(name="aux", bufs=2, space="PSUM") as aux,
            ):
                q0, k0 = phase1(0, aux, aux)
                nc.sync.dma_start(
                    out=wp_sb[:],
                    in_=wp_d.rearrange("(ct p) m -> p ct m", p=128))
                phase3(0, q0, k0, sp, oap, aux)
                q1, k1 = phase1(1, aux, None)
                phase3(1, q1, k1, sp, oap, aux)

    nc.compile()
    return nc


def rope_tables(h, w, n):
    """cos/sin lookup tables, tiled x4 along partitions -> [128, n]."""
    quarter = HD // 4  # 16
    pos_h, pos_w = np.meshgrid(np.arange(h, dtype=np.float64),
                               np.arange(w, dtype=np.float64), indexing="ij")
    pos = np.stack([pos_h.ravel(), pos_w.ravel()], axis=-1)[:n]
    freqs = 1.0 / (MAX_FREQ ** (np.arange(quarter, dtype=np.float64) / quarter))
    ang = np.concatenate([pos[:, 0:1] * freqs, pos[:, 1:2] * freqs], axis=-1)
    cos = np.cos(ang).T.astype(np.float32)  # [32, n]
    sin = np.sin(ang).T.astype(np.float32)
    return np.tile(cos, (4, 1)), np.tile(sin, (4, 1))


def host_prep(x, w_qkv, w_proj, n=4096, h=H, w=W):
    """Build the 8 per-core input maps."""
    x = np.asarray(x, dtype=np.float32)
    w_qkv = np.asarray(w_qkv, dtype=np.float32)
    w_proj = np.asarray(w_proj, dtype=np.float32)
    dim = x.shape[1]
    cos128, sin128 = rope_tables(h, w, n)

    def jmat(wh):  # wh [64, dim] -> J @ wh
        return np.concatenate([-wh[32:64], wh[0:32]], axis=0)

    in_maps = []
    for c in range(N_CORES):
        b, g = c // 2, c % 2
        hs = [4 * g + i for i in range(4)]
        cols = []
        for pair in range(2):
            h0, h1 = hs[2 * pair], hs[2 * pair + 1]
            wq0, wq1 = w_qkv[64 * h0:64 * h0 + 64], w_qkv[64 * h1:64 * h1 + 64]
            # 1/128 folded into k so scores arrive pre-scaled for the
            # poly-exp path (exp(S/8) = exp(16 * S/128))
            wk0 = w_qkv[dim + 64 * h0: dim + 64 * h0 + 64] / 128.0
            wk1 = w_qkv[dim + 64 * h1: dim + 64 * h1 + 64] / 128.0
            cols += [wq0, wq1, jmat(wq0), jmat(wq1),
                     wk0, wk1, jmat(wk0), jmat(wk1),
                     np.zeros((128, dim), np.float32)]  # v slot unused
        wqkvT = np.concatenate(cols, axis=0).T.copy()  # [dim, 1280]

        wvT = np.zeros((dim, 256), np.float32)
        for i, hh in enumerate(hs):
            wvT[:, 64 * i:64 * i + 64] = w_qkv[2 * dim + 64 * hh:
                                               2 * dim + 64 * hh + 64].T
        wprojT = w_proj[:, 256 * g:256 * g + 256].T.copy()  # [256, dim]

        in_maps.append({
            "x": np.ascontiguousarray(x[b].reshape(dim, n)).astype(np.float16),
            "wqkvT": np.ascontiguousarray(wqkvT).astype(np.float16),
            "wvT": wvT.astype(np.float16),
            "wprojT": np.ascontiguousarray(wprojT).astype(np.float16),
            "cos": cos128[:, :n].copy(),
            "sin": sin128[:, :n].copy(),
        })
    return in_maps


_NC_CACHE = {}


def kernel(x, w_qkv, w_proj, trace=False):
    key = "full"
    if key not in _NC_CACHE:
        _NC_CACHE[key] = build_nc(**FULL)
    nc = _NC_CACHE[key]
    in_maps = host_prep(x, w_qkv, w_proj)
    res = run_bass_kernel_spmd(nc, in_maps, list(range(N_CORES)), trace=trace)
    outs = [res.results[c]["out"] for c in range(N_CORES)]
    full = np.empty((B, DIM, H, W), np.float32)
    for b in range(B):
        full[b] = (outs[2 * b] + outs[2 * b + 1]).reshape(DIM, H, W)
    kernel.last_results = res
    return full

